# revision 1
# baseline (speedup 1.0000x reference)
"""BERT-CRF loss kernel for Trainium2 (8 NeuronCores, data-parallel over sentences).

Math: loss = sum_b(forward_b - cumsum(gold)_b) for a CRF whose forward scan runs
over the flattened B*S steps (batch carryover).  The log-semiring scan is
reassociated into per-chunk (L=16 positions) transfer matrices computed on
device in scaled probability space:

  feats[pos,t]   = hidden @ W.T + b          (PE, pos-major so the per-position
                                              max over live tags is a free-dim
                                              reduce and the exp bias is
                                              per-partition)
  EF = exp(feats - m)                         (ACT)
  chunk scan: A <- diag(EF_s) @ (E_live @ A)  (PE matmul + DVE broadcast-mul,
                                              bf16, rows = 10 live tags)

Host combines the 2048 tiny [10,12] chunk matrices sequentially in f64
(log-semiring matvec), reads off sentence-end vectors, and computes the gold
score from the shipped feats.  START/STOP rows are structurally zero in the
scan; their contributions are exactly 0 at float precision (e^-10000).

Per core: 8 sentences = 4096 positions; hidden arrives pre-transposed
[768, 4096] so the h-contraction sits on partitions.
"""
import numpy as np
import ml_dtypes
from contextlib import ExitStack

import concourse.bass as bass
import concourse.mybir as mybir
from concourse.tile import TileContext
from concourse.tile_rust import add_dep_helper
from concourse.bass_utils import run_bass_kernel_spmd

B, S, H, T = 64, 512, 768, 12
START, STOP, NEG = 10, 11, -10000.0
L = 16                   # chunk length (positions per transfer matrix)
NCORES = 8
P_CORE = B * S // NCORES  # 4096 positions per core
NCH = P_CORE // L         # 256 chunks per core
BF16 = ml_dtypes.bfloat16

F32 = mybir.dt.float32
BF = mybir.dt.bfloat16


def _build_nc():
    nc = bass.Bass()
    hiddenT = nc.declare_dram_parameter("hiddenT", [H, P_CORE], F32, isOutput=False)
    wt = nc.declare_dram_parameter("wt", [H, T], F32, isOutput=False)
    bvec = nc.declare_dram_parameter("bvec", [1, T], F32, isOutput=False)
    ones1 = nc.declare_dram_parameter("ones1", [1, 128], F32, isOutput=False)
    ident = nc.declare_dram_parameter("ident", [128, 128], BF, isOutput=False)
    etl = nc.declare_dram_parameter("etl", [128, 32], BF, isOutput=False)
    e40 = nc.declare_dram_parameter("e40", [96, 384], BF, isOutput=False)
    zeros = nc.declare_dram_parameter("zeros", [96, 512], BF, isOutput=False)
    feats_pm = nc.declare_dram_parameter("feats_pm", [P_CORE, T], F32, isOutput=True)
    m_out = nc.declare_dram_parameter("m_out", [128, 32], F32, isOutput=True)
    a_out = nc.declare_dram_parameter("a_out", [288, 384], BF, isOutput=True)

    SG_OF = [0, 0, 0, 1, 1, 1, 2, 2]
    SLOT_OF = [0, 1, 2, 0, 1, 2, 0, 1]
    last_insts = {}
    out_dmas = []

    with ExitStack() as ctx:
        tc = ctx.enter_context(TileContext(nc))
        const_pool = ctx.enter_context(tc.tile_pool(name="const", bufs=1))
        hid_pool = ctx.enter_context(tc.tile_pool(name="hid", bufs=48))
        mneg_pool = ctx.enter_context(tc.tile_pool(name="mneg", bufs=32))
        ef_pool = ctx.enter_context(tc.tile_pool(name="efp", bufs=32))
        a_pool = ctx.enter_context(tc.tile_pool(name="apool", bufs=48))
        psf_pool = ctx.enter_context(tc.tile_pool(name="psf", bufs=2, space="PSUM"))
        pst_pool = ctx.enter_context(tc.tile_pool(name="pst", bufs=2, space="PSUM"))
        pss_pool = ctx.enter_context(tc.tile_pool(name="pss", bufs=3, space="PSUM"))
        warm_pool = ctx.enter_context(tc.tile_pool(name="warm", bufs=1, space="PSUM"))

        # ---- constants (each DMA has no deps -> 0 waits) ----
        wt_sb = const_pool.tile([128, 6 * T], F32)
        nc.gpsimd.dma_start(
            out=wt_sb[:, :].rearrange("p (k t) -> p k t", t=T),
            in_=wt[:, :].rearrange("(k p) t -> p k t", p=128),
        )
        b_sb = const_pool.tile([1, T], F32)
        nc.gpsimd.dma_start(out=b_sb[:, :], in_=bvec[:, :])
        ones_sb = const_pool.tile([1, 128], F32)
        nc.gpsimd.dma_start(out=ones_sb[:, :], in_=ones1[:, :])
        ident_sb = const_pool.tile([128, 128], BF)
        nc.gpsimd.dma_start(out=ident_sb[:, :], in_=ident[:, :])
        etl_sb = const_pool.tile([128, 32], BF)
        nc.gpsimd.dma_start(out=etl_sb[:, :], in_=etl[:, :])
        e40_sb = const_pool.tile([96, 384], BF)
        nc.gpsimd.dma_start(out=e40_sb[:, :], in_=e40[:, :])
        m_all = const_pool.tile([128, 32], F32)
        feats_big = const_pool.tile([128, 384], F32)
        ef40_tiles = [const_pool.tile([96, 512], BF, name=f"ef40_{i}",
                                      tag=f"ef40_{i}") for i in range(3)]
        for i in range(3):
            nc.gpsimd.dma_start(out=ef40_tiles[i][:, :], in_=zeros[:, :])

        # ---- warm-up touches: after these, no instruction needs more than
        # one semaphore wait (ISA sync-slot limit on LDW / DMA descriptors).
        wp = warm_pool.tile([128, 128], F32)
        nc.tensor.matmul(wp[0:12, 0:12], lhsT=wt_sb[:, 0:T], rhs=wt_sb[:, 0:T],
                         start=True, stop=True)
        nc.tensor.matmul(wp[0:128, 0:12], lhsT=ones_sb[:, :],
                         rhs=ones_sb[0:1, 0:T], start=True, stop=True)
        nc.tensor.matmul(wp[0:12, 0:12], lhsT=b_sb[:, :], rhs=b_sb[:, :],
                         start=True, stop=True)
        nc.tensor.matmul(wp[0:32, 0:32], lhsT=etl_sb[0:10, :],
                         rhs=etl_sb[0:10, :], start=True, stop=True)
        nc.tensor.matmul(wp[0:128, 0:1], lhsT=ident_sb[:, :],
                         rhs=ident_sb[:, 0:1], start=True, stop=True)
        scr_v = const_pool.tile([1, 8], BF)
        nc.vector.tensor_copy(scr_v[0:1, 0:1], e40_sb[0:1, 0:1])
        scr_a = const_pool.tile([1, 8], F32)
        for i in range(3):
            nc.vector.tensor_copy(scr_v[0:1, 4 + i:5 + i],
                                  ef40_tiles[i][0:1, 0:1])
            nc.scalar.activation(scr_a[0:1, i:i + 1], ef40_tiles[i][0:1, 0:1],
                                 mybir.ActivationFunctionType.Copy)

        # ---- input stream: 48 distinct tiles, no reuse -> 0-wait DMAs ----
        hid_sb = {}
        in_dmas = []
        for g in range(8):
            for hs in range(6):
                t = hid_pool.tile([128, 512], F32, name=f"hid_{g}_{hs}", tag="hid")
                di = nc.gpsimd.dma_start(
                    out=t[:, :],
                    in_=hiddenT[hs * 128:(hs + 1) * 128, g * 512:(g + 1) * 512],
                )
                in_dmas.append(di)
                hid_sb[(g, hs)] = t

        def feats_block(g):
            sg, slot = SG_OF[g], SLOT_OF[g]
            nc.tensor.matmul(wp[0:1, 0:1], lhsT=hid_sb[(g, 0)][:, 0:1],
                             rhs=hid_sb[(g, 0)][:, 0:1], start=True, stop=True)
            for pt in range(4):
                col = g * 4 + pt
                psf = psf_pool.tile([128, T], F32)
                for hs in range(6):
                    nc.tensor.matmul(
                        psf[:, :],
                        lhsT=hid_sb[(g, hs)][:, pt * 128:(pt + 1) * 128],
                        rhs=wt_sb[:, hs * T:(hs + 1) * T],
                        start=(hs == 0), stop=False,
                    )
                nc.tensor.matmul(
                    psf[:, :], lhsT=ones_sb[:, :], rhs=b_sb[:, :],
                    start=False, stop=True,
                )
                nc.scalar.activation(
                    feats_big[:, col * T:(col + 1) * T], psf[:, :],
                    mybir.ActivationFunctionType.Copy)
                nc.vector.reduce_max(
                    out=m_all[:, col:col + 1],
                    in_=feats_big[:, col * T:col * T + 10],
                    axis=mybir.AxisListType.X,
                )
                mneg = mneg_pool.tile([128, 1], F32, name=f"mneg_{col}", tag="mneg")
                nc.scalar.activation(
                    mneg[:, :], m_all[:, col:col + 1],
                    mybir.ActivationFunctionType.Copy, scale=-1.0,
                )
                ef_pos = ef_pool.tile([128, T], BF, name=f"efpos_{col}", tag="efpos")
                nc.scalar.activation(
                    ef_pos[:, :], psf[:, :], mybir.ActivationFunctionType.Exp,
                    bias=mneg[:, 0:1], scale=1.0,
                )
                pst = pst_pool.tile([T, 128], BF)
                nc.tensor.transpose(pst[:, :], ef_pos[:, :], ident_sb[:, :])
                ai = nc.scalar.activation(
                    ef40_tiles[sg][slot * 32:slot * 32 + 10,
                                   pt * 128:(pt + 1) * 128],
                    pst[0:10, :], mybir.ActivationFunctionType.Copy,
                )
                last_insts["act"] = ai

        def scan_sg(sg):
            nslots = 3
            ef40 = ef40_tiles[sg]
            # absorb the ACT (EF writes) wait into the DVE clock up front
            nc.vector.tensor_copy(scr_v[0:1, 1 + sg:2 + sg], ef40[0:1, 0:1])
            At = None
            for s in range(16):
                ef_base = ef40[:, s::16]            # [128, 32] (chunk stride L)
                ef_ap = bass.AP(ef_base.tensor, ef_base.offset,
                                list(ef_base.ap) + [[0, T]])  # [128, 32, 12]
                At2 = a_pool.tile([96, 384], BF, name=f"at_{sg}_{s}", tag="at")
                if s == 0:
                    nc.vector.tensor_mul(
                        At2[:, :].rearrange("p (c j) -> p c j", j=T),
                        e40_sb[:, :].rearrange("p (c j) -> p c j", j=T),
                        ef_ap,
                    )
                else:
                    # absorber 1: pull the DVE (At ready) tick into PE clock
                    ab1 = nc.tensor.matmul(wp[0:1, 0:1], lhsT=At[0:1, 0:1],
                                           rhs=At[0:1, 0:1], start=True,
                                           stop=True)
                    ps = pss_pool.tile([96, 384], F32)
                    # absorber 2: dummy first-writer carries the PSUM
                    # bank-reuse hazard wait
                    ab2 = nc.tensor.matmul(ps[0:1, 0:1], lhsT=etl_sb[0:1, 0:1],
                                           rhs=etl_sb[0:1, 0:1],
                                           start=True, stop=True,
                                           skip_group_check=True)
                    add_dep_helper(ab2.ins, ab1.ins, False,
                                   "absorber ordering")
                    for u in range(nslots):
                        mi = nc.tensor.matmul(
                            ps[u * 32:(u + 1) * 32, :],
                            lhsT=etl_sb[u * 32:u * 32 + 10, :],
                            rhs=At[u * 32:u * 32 + 10, :],
                            start=True, stop=True,
                            skip_group_check=True,
                        )
                        last_insts["pe"] = mi
                    vi = nc.vector.tensor_mul(
                        At2[:, :].rearrange("p (c j) -> p c j", j=T),
                        ps[:, :].rearrange("p (c j) -> p c j", j=T),
                        ef_ap,
                    )
                    last_insts["dve"] = vi
                At = At2
            oi = nc.sync.dma_start(out=a_out[sg * 96:(sg + 1) * 96, :],
                                    in_=At[:, :])
            out_dmas.append(oi)

        for g in range(3):
            feats_block(g)
        scan_sg(0)
        for g in range(3, 6):
            feats_block(g)
        scan_sg(1)
        for g in range(6, 8):
            feats_block(g)
        scan_sg(2)
        oi = nc.sync.dma_start(
            out=feats_pm[:, :].rearrange("(c p) t -> p c t", p=128),
            in_=feats_big[:, :].rearrange("p (c t) -> p c t", t=T),
        )
        out_dmas.append(oi)
        oi = nc.sync.dma_start(out=m_out[:, :], in_=m_all[:, :])
        out_dmas.append(oi)
        # Pre-absorb every proc's clock into SP one dep at a time, so the
        # Tile tail drain does not need a multi-sem wait.
        for dep in in_dmas[-8:] + out_dmas + list(last_insts.values()):
            nop = nc.sync.nop()
            add_dep_helper(nop.ins, dep.ins, True, "drain preclear")
    return nc


_NC_CACHE = None


def _get_nc():
    global _NC_CACHE
    if _NC_CACHE is None:
        _NC_CACHE = _build_nc()
    return _NC_CACHE


def _build_etl128(E):
    e = np.zeros((128, 32), np.float32)
    for slot in range(3):
        e[slot * 32:slot * 32 + 10, 0:10] = E[:10, :10].T
    return e.astype(BF16)


def _build_e128(E):
    e = np.zeros((96, 384), np.float32)
    for slot in range(3):
        e[slot * 32:slot * 32 + 10, :] = np.tile(E[:10, :], (1, 32))
    return e.astype(BF16)


def _run_device(hidden, W, b, transitions, trace=False, tmpdir=None):
    E = np.exp(transitions.astype(np.float64))
    E[START, :] = 0.0            # structurally dead (no transition out of a
    E[STOP, :] = 0.0             # state that can't receive / into one that
    E[:, STOP] = 0.0             # can't send): contributions are e^-10000 = 0.
    E = E.astype(np.float32)

    wt_np = np.ascontiguousarray(W.T).astype(np.float32)
    in_common = {
        "wt": wt_np,
        "bvec": b.reshape(1, T).astype(np.float32),
        "ones1": np.ones((1, 128), np.float32),
        "ident": np.eye(128).astype(BF16),
        "etl": _build_etl128(E),
        "e40": _build_e128(E),
        "zeros": np.zeros((96, 512), BF16),
    }
    flat = hidden.reshape(B * S, H)
    in_maps = []
    for c in range(NCORES):
        hT = np.ascontiguousarray(
            flat[c * P_CORE:(c + 1) * P_CORE].T).astype(np.float32)
        d = dict(in_common)
        d["hiddenT"] = hT
        in_maps.append(d)

    res = run_bass_kernel_spmd(
        _get_nc(), in_maps, list(range(NCORES)), trace=trace, tmpdir=tmpdir)
    return res


def _host_combine(results, transitions, tags):
    feats = np.concatenate(
        [np.asarray(r["feats_pm"]) for r in results], axis=0)  # [B*S, T] f32
    m_flat = np.concatenate(
        [np.asarray(r["m_out"]).T.reshape(P_CORE) for r in results])  # [B*S]
    # chunk matrices [2048, 10, 12]
    A = np.zeros((NCORES * NCH, 10, T), np.float32)
    for c, r in enumerate(results):
        a = np.asarray(r["a_out"]).astype(np.float32)  # [288, 384]
        SG_OF = [0, 0, 0, 1, 1, 1, 2, 2]
        SLOT_OF = [0, 1, 2, 0, 1, 2, 0, 1]
        for g in range(8):
            sg, slot = SG_OF[g], SLOT_OF[g]
            blk = a[sg * 96 + slot * 32: sg * 96 + slot * 32 + 10, :]  # [10,384]
            A[c * NCH + g * 32:c * NCH + (g + 1) * 32] = (
                blk.reshape(10, 32, T).transpose(1, 0, 2))
    n_chunks = NCORES * NCH
    scale = m_flat.astype(np.float64).reshape(n_chunks, L).sum(axis=1)
    with np.errstate(divide="ignore"):
        logP = np.log(A.astype(np.float64)) + scale[:, None, None]

    v = np.full(T, NEG, np.float64)
    v[START] = 0.0
    last = np.zeros((B, T), np.float64)
    cps = S // L
    err = np.errstate(invalid="ignore", divide="ignore", over="ignore")
    err.__enter__()
    for c in range(n_chunks):
        x = logP[c] + v[None, :]
        mx = np.max(x, axis=1)
        mx_safe = np.where(np.isfinite(mx), mx, 0.0)
        with np.errstate(invalid="ignore"):
            vl = mx + np.log(np.sum(np.exp(x - mx_safe[:, None]), axis=1))
        vl = np.where(np.isfinite(mx), vl, -np.inf)
        v = np.concatenate([vl, [-np.inf, -np.inf]])
        if (c + 1) % cps == 0:
            last[(c + 1) // cps - 1] = v
    x = last + transitions[STOP][None, :].astype(np.float64)
    mx = x.max(axis=1)
    forward_score = mx + np.log(np.exp(x - mx[:, None]).sum(axis=1))  # [B]
    err.__exit__(None, None, None)

    tags_ext = np.concatenate(
        [np.full((B, 1), START, dtype=tags.dtype), tags], axis=1)
    prev, nxt = tags_ext[:, :-1], tags_ext[:, 1:]
    trans_sc = transitions[nxt, prev].astype(np.float64).sum(axis=1)
    featsb = feats.reshape(B, S, T)
    emit_sc = np.take_along_axis(
        featsb.astype(np.float64), nxt[..., None].astype(np.int64), axis=2
    )[..., 0].sum(axis=1)
    gold = trans_sc + emit_sc + transitions[STOP, tags_ext[:, -1]].astype(np.float64)
    gold_cum = np.cumsum(gold)
    out = np.sum(forward_score - gold_cum)
    return np.array([out], dtype=np.float32)


def kernel(hidden, W, b, transitions, tags, _trace=False, _tmpdir=None):
    hidden = np.asarray(hidden, dtype=np.float32)
    W = np.asarray(W, dtype=np.float32)
    b = np.asarray(b, dtype=np.float32)
    transitions = np.asarray(transitions, dtype=np.float32)
    tags = np.asarray(tags)
    res = _run_device(hidden, W, b, transitions, trace=_trace, tmpdir=_tmpdir)
    out = _host_combine(res.results, transitions, tags)
    if _trace:
        return out, res
    return out



# revision 4
# speedup vs baseline: 3.3999x; 3.3999x over previous
"""BERT-CRF loss kernel for Trainium2 (8 NeuronCores, data-parallel over positions).

Math: loss = sum_b(forward_b - cumsum(gold)_b) for a CRF whose forward scan runs
over the flattened B*S steps (batch carryover).  The log-semiring scan is
reassociated into per-chunk (L=4 positions) transfer matrices computed on
device in scaled probability space:

  feats[pos,t]  = hidden @ W.T            (PE, pos-major: tags on the free dim
                                           so each matmul costs only 12 rows;
                                           the bias b is folded into the
                                           transition matrix E' = diag(e^b) E
                                           on the host, so no bias add at all)
  mneg[pos]     = -max_t feats[pos,0:10]  (DVE reduce, batched 4 tiles/op)
  EF            = exp(feats - m)          (ACT, bias = mneg per partition)
  chunk scan    : A <- diag(EF_s) @ (E' @ A)

The scan runs in 4 independent passes of 4 sequential steps each. A pass
covers 8 position tiles (slots) stacked 12 rows apiece on 96 partitions; the
E' matmul is ONE block-diagonal [96x96] @ [96x384] per step, and the diag(EF)
is ONE DVE broadcast-multiply. EF is transposed tag-major with a single PE
transpose [128,96]->[96,128] per pass. Dead tags (START/STOP) ride along as
structurally-zero rows.

Host combines the 8192 tiny [12,12] chunk matrices sequentially in f64
(log-semiring matvec), reads off sentence-end vectors, and computes the gold
score from the shipped feats.

Per core: 8 sentences = 4096 positions; hidden arrives pre-transposed
[768, 4096] (fp8 e4m3 by default) so the h-contraction sits on partitions.
"""
import numpy as np
import ml_dtypes
from contextlib import ExitStack

import concourse.bass as bass
import concourse.mybir as mybir
from concourse.tile import TileContext
from concourse.tile_rust import add_dep_helper
from concourse.bass_utils import run_bass_kernel_spmd

B, S, H, T = 64, 512, 768, 12
START, STOP, NEG = 10, 11, -10000.0
NCORES = 8
P_CORE = B * S // NCORES     # 4096 positions per core
L = 4                        # chunk length (positions per transfer matrix)
NCH = P_CORE // L            # 1024 chunks per core
NPASS = 4                    # scan passes per core
TPP = 8                      # position tiles (slots) per pass
NBLK = 8                     # input DMA blocks (512 positions each)
CPS = 128 // L               # chunks per slot (32)

BF16 = ml_dtypes.bfloat16
FP8 = ml_dtypes.float8_e4m3
F32 = mybir.dt.float32
BF = mybir.dt.bfloat16

HID_DT = mybir.dt.float8e4   # device dtype for hidden/W
HID_NP = FP8                 # matching numpy dtype


def _build_nc():
    nc = bass.Bass()
    hiddenT = nc.declare_dram_parameter("hiddenT", [H, P_CORE], HID_DT,
                                        isOutput=False)
    wt = nc.declare_dram_parameter("wt", [H, T], HID_DT, isOutput=False)
    bd = nc.declare_dram_parameter("bd", [96, 96], BF, isOutput=False)
    e40 = nc.declare_dram_parameter("e40", [96, 384], BF, isOutput=False)
    ident = nc.declare_dram_parameter("ident", [128, 128], BF, isOutput=False)
    feats_out = nc.declare_dram_parameter("feats_out", [128, 384], BF,
                                          isOutput=True)
    m_out = nc.declare_dram_parameter("m_out", [128, 32], F32, isOutput=True)
    a_out = nc.declare_dram_parameter("a_out", [NPASS * 96, 384], BF,
                                      isOutput=True)

    last = {}
    out_dmas = []
    in_dmas = []

    with ExitStack() as ctx:
        tc = ctx.enter_context(TileContext(nc))
        const_pool = ctx.enter_context(tc.tile_pool(name="const", bufs=1))
        hid_pool = ctx.enter_context(tc.tile_pool(name="hid", bufs=NBLK))
        efpm_pool = ctx.enter_context(tc.tile_pool(name="efpm", bufs=NPASS))
        ef40_pool = ctx.enter_context(tc.tile_pool(name="ef40", bufs=NPASS))
        a_pool = ctx.enter_context(tc.tile_pool(name="apool", bufs=16))
        psf_pool = ctx.enter_context(tc.tile_pool(name="psf", bufs=2,
                                                  space="PSUM"))
        pst_pool = ctx.enter_context(tc.tile_pool(name="pst", bufs=2,
                                                  space="PSUM"))
        pss_pool = ctx.enter_context(tc.tile_pool(name="pss", bufs=2,
                                                  space="PSUM"))
        warm_pool = ctx.enter_context(tc.tile_pool(name="warm", bufs=1,
                                                   space="PSUM"))

        # ---- constants ----
        wt_sb = const_pool.tile([128, 6 * T], HID_DT)
        nc.sync.dma_start(
            out=wt_sb[:, :].rearrange("p (k t) -> p k t", t=T),
            in_=wt[:, :].rearrange("(k p) t -> p k t", p=128),
        )
        bd_sb = const_pool.tile([96, 96], BF)
        nc.sync.dma_start(out=bd_sb[:, :], in_=bd[:, :])
        e40_sb = const_pool.tile([96, 384], BF)
        nc.sync.dma_start(out=e40_sb[:, :], in_=e40[:, :])
        ident_sb = const_pool.tile([128, 128], BF)
        nc.sync.dma_start(out=ident_sb[:, :], in_=ident[:, :])

        feats_big = const_pool.tile([128, 384], BF)
        m_all = const_pool.tile([128, 32], F32)

        # ---- input stream: 8 block DMAs, no deps -> 0-wait descriptors ----
        hid_sb = []
        for blk in range(NBLK):
            t = hid_pool.tile([128, 6 * 512], HID_DT, name=f"hid_{blk}",
                              tag="hid")
            di = nc.sync.dma_start(
                out=t[:, :].rearrange("p (k c) -> p k c", c=512),
                in_=hiddenT[:, blk * 512:(blk + 1) * 512].rearrange(
                    "(k p) c -> p k c", p=128),
            )
            in_dmas.append(di)
            hid_sb.append(t)

        # ---- warm-up touches: absorb const-DMA waits + load the ACT
        # exp/copy table once.
        wp = warm_pool.tile([128, 128], F32)
        nc.tensor.matmul(wp[0:T, 0:T], lhsT=wt_sb[:, 0:T], rhs=wt_sb[:, 0:T],
                         start=True, stop=True)
        nc.tensor.matmul(wp[0:96, 0:96], lhsT=bd_sb[:, :], rhs=bd_sb[:, :],
                         start=True, stop=True)
        nc.tensor.matmul(wp[0:128, 0:1], lhsT=ident_sb[:, :],
                         rhs=ident_sb[:, 0:1], start=True, stop=True)
        scr_v = const_pool.tile([1, 8], BF)
        nc.vector.tensor_copy(scr_v[0:1, 0:1], e40_sb[0:1, 0:1])
        scr_a = const_pool.tile([1, 8], F32)
        nc.scalar.activation(scr_a[0:1, 0:1], scr_v[0:1, 0:1],
                             mybir.ActivationFunctionType.Exp)

        ef_pm = [efpm_pool.tile([128, 96], BF, name=f"efpm_{p}", tag="efpm")
                 for p in range(NPASS)]
        ef40 = [ef40_pool.tile([96, 128], BF, name=f"ef40_{p}", tag="ef40")
                for p in range(NPASS)]

        def feats_block(blk):
            """One 512-position block: 24 matmuls, 1 reduce, 1 copy, 4 exps."""
            psf = psf_pool.tile([128, 48], F32)
            for pt in range(4):
                for k in range(6):
                    nc.tensor.matmul(
                        psf[:, pt * T:(pt + 1) * T],
                        lhsT=hid_sb[blk][:, k * 512 + pt * 128:
                                         k * 512 + (pt + 1) * 128],
                        rhs=wt_sb[:, k * T:(k + 1) * T],
                        start=(k == 0), stop=(k == 5),
                        skip_group_check=True,
                    )
            # -max over live tags, 4 tiles at once
            ri = nc.vector.tensor_reduce(
                out=m_all[:, blk * 4:(blk + 1) * 4],
                in_=psf[:, :].rearrange("p (g t) -> p g t", t=T)[:, :, 0:10],
                op=mybir.AluOpType.max, axis=mybir.AxisListType.X,
                negate=True,
            )
            last["red"] = ri
            ci = nc.scalar.activation(
                feats_big[:, blk * 48:(blk + 1) * 48], psf[:, :],
                mybir.ActivationFunctionType.Copy)
            last["fcopy"] = ci
            for pt in range(4):
                tt = blk * 4 + pt
                p, slot = tt // TPP, tt % TPP
                ei = nc.scalar.activation(
                    ef_pm[p][:, slot * T:slot * T + 10],
                    psf[:, pt * T:pt * T + 10],
                    mybir.ActivationFunctionType.Exp,
                    bias=m_all[:, tt:tt + 1], scale=1.0,
                )
                last["exp"] = ei

        def scan_pass(p):
            # zero the dead-tag columns (ACT, in-order before this pass's exps
            # have all retired is fine: cols 10,11 are never written by exps)
            nc.gpsimd.memset(
                ef_pm[p][:, :].rearrange("p (s t) -> p s t", t=T)[:, :, 10:12],
                0.0)

        def scan_pass_body(p):
            pst = pst_pool.tile([96, 128], BF, name=f"pst_{p}", tag="pst")
            nc.tensor.transpose(pst[:, :], ef_pm[p][:, :], ident_sb[:, 0:128])
            cpi = nc.vector.tensor_copy(ef40[p][:, :], pst[:, :])
            At = None
            for s in range(L):
                ef_base = ef40[p][:, s::L]               # [96, 32]
                ef_ap = bass.AP(ef_base.tensor, ef_base.offset,
                                list(ef_base.ap) + [[0, T]])  # [96, 32, 12]
                At2 = a_pool.tile([96, 384], BF, name=f"at_{p}_{s}", tag="at")
                if s == 0:
                    vi = nc.vector.tensor_mul(
                        At2[:, :].rearrange("p (c j) -> p c j", j=T),
                        e40_sb[:, :].rearrange("p (c j) -> p c j", j=T),
                        ef_ap,
                    )
                else:
                    ps = pss_pool.tile([96, 384], F32)
                    mi = nc.tensor.matmul(ps[:, :], lhsT=bd_sb[:, :],
                                          rhs=At[:, :], start=True, stop=True)
                    last["pe"] = mi
                    vi = nc.vector.tensor_mul(
                        At2[:, :].rearrange("p (c j) -> p c j", j=T),
                        ps[:, :].rearrange("p (c j) -> p c j", j=T),
                        ef_ap,
                    )
                last["dve"] = vi
                At = At2
            oi = nc.sync.dma_start(out=a_out[p * 96:(p + 1) * 96, :],
                                   in_=At[:, :])
            out_dmas.append(oi)

        for p in range(NPASS):
            scan_pass(p)          # dead-col memsets early (ACT, cheap)
        for blk in range(NBLK):
            feats_block(blk)
            if blk % 2 == 1:
                scan_pass_body(blk // 2)

        oi = nc.scalar.dma_start(out=feats_out[:, :], in_=feats_big[:, :])
        out_dmas.append(oi)
        oi = nc.scalar.dma_start(out=m_out[:, :], in_=m_all[:, :])
        out_dmas.append(oi)
        # Pre-absorb every proc's clock into SP one dep at a time, so the
        # Tile tail drain does not need a multi-sem wait.
        for dep in in_dmas[-2:] + out_dmas + list(last.values()):
            nop = nc.sync.nop()
            add_dep_helper(nop.ins, dep.ins, True, "drain preclear")
    return nc


_NC_CACHE = None


def _get_nc():
    global _NC_CACHE
    if _NC_CACHE is None:
        _NC_CACHE = _build_nc()
    return _NC_CACHE


def _build_eprime(transitions, b):
    """E' = diag(e^b) exp(transitions) with structurally-dead rows/cols zeroed."""
    E = np.exp(transitions.astype(np.float64))
    E[START, :] = 0.0
    E[STOP, :] = 0.0
    E[:, STOP] = 0.0
    E = E * np.exp(b.astype(np.float64))[:, None]
    return E


def _build_bd(Ep):
    """Block-diagonal stationary operand: bd[slot*12+p, slot*12+t'] = E'[t',p]."""
    bd = np.zeros((96, 96), np.float64)
    for s in range(TPP):
        bd[s * T:(s + 1) * T, s * T:(s + 1) * T] = Ep.T
    return bd.astype(BF16)


def _build_e40(Ep):
    """Step-0 init: e40[slot*12+t, c*12+j] = E'[t, j]."""
    e = np.zeros((96, 384), np.float64)
    tile = np.tile(Ep, (1, CPS))
    for s in range(TPP):
        e[s * T:(s + 1) * T, :] = tile
    return e.astype(BF16)


def _sim_input_map(inputs, core):
    """Per-core device input map (also used by test harnesses)."""
    hidden = np.asarray(inputs["hidden"], dtype=np.float32)
    W = np.asarray(inputs["W"], dtype=np.float32)
    b = np.asarray(inputs["b"], dtype=np.float32)
    transitions = np.asarray(inputs["transitions"], dtype=np.float32)
    Ep = _build_eprime(transitions, b)
    flat = hidden.reshape(B * S, H)
    hT = np.ascontiguousarray(
        flat[core * P_CORE:(core + 1) * P_CORE].T).astype(HID_NP)
    return {
        "hiddenT": hT,
        "wt": np.ascontiguousarray(W.T).astype(HID_NP),
        "bd": _build_bd(Ep),
        "e40": _build_e40(Ep),
        "ident": np.eye(128).astype(BF16),
    }


def _run_device(hidden, W, b, transitions, trace=False, tmpdir=None):
    Ep = _build_eprime(transitions, b)
    in_common = {
        "wt": np.ascontiguousarray(W.T).astype(HID_NP),
        "bd": _build_bd(Ep),
        "e40": _build_e40(Ep),
        "ident": np.eye(128).astype(BF16),
    }
    flat = hidden.reshape(B * S, H)
    in_maps = []
    for c in range(NCORES):
        hT = np.ascontiguousarray(
            flat[c * P_CORE:(c + 1) * P_CORE].T).astype(HID_NP)
        d = dict(in_common)
        d["hiddenT"] = hT
        in_maps.append(d)

    res = run_bass_kernel_spmd(
        _get_nc(), in_maps, list(range(NCORES)), trace=trace, tmpdir=tmpdir)
    return res


def _host_combine(results, transitions, b, tags):
    # feats [B*S, T] (bf16 -> f64), WITHOUT the bias b
    feats = np.concatenate([
        np.asarray(r["feats_out"]).astype(np.float64)
        .reshape(128, 32, T).transpose(1, 0, 2).reshape(P_CORE, T)
        for r in results], axis=0)
    # m per position (device ships -m)
    m_flat = np.concatenate([
        -np.asarray(r["m_out"]).astype(np.float64).T.reshape(P_CORE)
        for r in results])
    # chunk matrices [NCORES*NCH, 12, 12]
    A = np.concatenate([
        np.asarray(r["a_out"]).astype(np.float64)
        .reshape(NPASS, TPP, T, CPS, T).transpose(0, 1, 3, 2, 4)
        .reshape(NCH, T, T)
        for r in results], axis=0)
    n_chunks = NCORES * NCH
    scale = m_flat.reshape(n_chunks, L).sum(axis=1)
    with np.errstate(divide="ignore"):
        logP = np.log(A) + scale[:, None, None]

    v = np.full(T, NEG, np.float64)
    v[START] = 0.0
    last = np.zeros((B, T), np.float64)
    cps_sentence = S // L
    err = np.errstate(invalid="ignore", divide="ignore", over="ignore")
    err.__enter__()
    for c in range(n_chunks):
        x = logP[c] + v[None, :]
        mx = np.max(x, axis=1)
        mx_safe = np.where(np.isfinite(mx), mx, 0.0)
        vl = mx + np.log(np.sum(np.exp(x - mx_safe[:, None]), axis=1))
        v = np.where(np.isfinite(mx), vl, -np.inf)
        if (c + 1) % cps_sentence == 0:
            last[(c + 1) // cps_sentence - 1] = v
    x = last + transitions[STOP][None, :].astype(np.float64)
    mx = x.max(axis=1)
    forward_score = mx + np.log(np.exp(x - mx[:, None]).sum(axis=1))  # [B]
    err.__exit__(None, None, None)

    tags_ext = np.concatenate(
        [np.full((B, 1), START, dtype=tags.dtype), tags], axis=1)
    prev, nxt = tags_ext[:, :-1], tags_ext[:, 1:]
    trans_sc = transitions[nxt, prev].astype(np.float64).sum(axis=1)
    featsb = feats.reshape(B, S, T)
    emit_sc = np.take_along_axis(
        featsb, nxt[..., None].astype(np.int64), axis=2)[..., 0].sum(axis=1)
    emit_sc = emit_sc + b.astype(np.float64)[nxt].sum(axis=1)
    gold = trans_sc + emit_sc + transitions[STOP, tags_ext[:, -1]].astype(np.float64)
    gold_cum = np.cumsum(gold)
    out = np.sum(forward_score - gold_cum)
    return np.array([out], dtype=np.float32)


def kernel(hidden, W, b, transitions, tags, _trace=False, _tmpdir=None):
    hidden = np.asarray(hidden, dtype=np.float32)
    W = np.asarray(W, dtype=np.float32)
    b = np.asarray(b, dtype=np.float32)
    transitions = np.asarray(transitions, dtype=np.float32)
    tags = np.asarray(tags)
    res = _run_device(hidden, W, b, transitions, trace=_trace, tmpdir=_tmpdir)
    out = _host_combine(res.results, transitions, b, tags)
    if _trace:
        return out, res
    return out
